# revision 17
# baseline (speedup 1.0000x reference)
"""3-layer GraphSAGE (mean aggregation) on 8 TRN2 NeuronCores.

Self-contained: hardcoded problem shapes (N=50000, E=800000, H=256, 3 layers).

Strategy
--------
Host side (numpy, inside kernel()):
  * degree-balanced assignment of nodes to 8 cores x 49 blocks of <=128 nodes
  * permuted "table" layout: table row = core*6272 + block*128 + pos
  * per-block edge lists as int16 gather indices (split at row 32768 because
    dma_gather indices are signed int16) + per-edge segment ids
  * everything padded to uniform subtile counts so all 8 cores run one SPMD
    program (padding gathers row 0 with segment id 200 -> zero one-hot column)

Device side (Bass/Tile), per layer:
  gather messages (bf16) -> one-hot via DVE is_equal -> segmented sum via
  TensorE matmuls accumulating in PSUM -> 1/deg scale -> PE transposes ->
  dense matmuls against W^T (+ K=1 bias matmul) -> ELU -> shard write ->
  AllGather (bf16) between layers.
"""

import sys

sys.path.insert(0, "/opt/trn_rl_repo")

import numpy as np
import ml_dtypes

from concourse import bacc, bass, mybir, tile
from concourse.bass_utils import run_bass_kernel_spmd
from concourse.masks import make_identity

bf16 = ml_dtypes.bfloat16

N_NODES = 50000
N_EDGES = 800000
H = 256
NC = 8
P = 128
NB = 49                      # blocks per core
STRIDE = NB * P              # 6272 table rows per core
TAB = NC * STRIDE            # 50176 table rows
HI_BASE = 32768              # int16 index split point


def _assign_blocks(deg: np.ndarray) -> np.ndarray:
    """Serpentine deal of nodes (sorted by degree desc) into NC*NB blocks.

    Returns block_of_node [N] in 0..NC*NB-1. Each block gets <=128 nodes and
    near-equal total degree.
    """
    nb_total = NC * NB
    order = np.argsort(-deg, kind="stable")
    block_of_node = np.empty(N_NODES, dtype=np.int64)
    pos = 0
    rnd = 0
    while pos < N_NODES:
        take = min(nb_total, N_NODES - pos)
        blocks = np.arange(nb_total) if rnd % 2 == 0 else np.arange(nb_total)[::-1]
        block_of_node[order[pos:pos + take]] = blocks[:take]
        pos += take
        rnd += 1
    return block_of_node


def _preprocess(edge_index: np.ndarray):
    """Graph preprocessing. Returns dict of host-side structures."""
    src = np.asarray(edge_index[0], dtype=np.int64)
    dst = np.asarray(edge_index[1], dtype=np.int64)
    deg = np.bincount(dst, minlength=N_NODES).astype(np.int64)

    block_of_node = _assign_blocks(deg)

    # position of each node within its block; table row of each node
    order = np.lexsort((np.arange(N_NODES), block_of_node))
    pos_in_block = np.empty(N_NODES, dtype=np.int64)
    counts = np.zeros(NC * NB, dtype=np.int64)
    for n in order:
        b = block_of_node[n]
        pos_in_block[n] = counts[b]
        counts[b] += 1
    assert counts.max() <= P, f"block overflow: {counts.max()}"
    table_row = block_of_node * P + pos_in_block  # global block index * 128 + pos

    # edges grouped by destination block
    e_block = block_of_node[dst]                  # block of each edge's dst
    e_seg = pos_in_block[dst]                     # segment id within block
    e_srcrow = table_row[src]                     # gather row of source

    sort_idx = np.argsort(e_block, kind="stable")
    e_block_s = e_block[sort_idx]
    e_seg_s = e_seg[sort_idx]
    e_srcrow_s = e_srcrow[sort_idx]
    blk_starts = np.searchsorted(e_block_s, np.arange(NC * NB + 1))

    # lo/hi split sizes per block -> uniform subtile counts
    lo_counts = np.empty(NC * NB, dtype=np.int64)
    hi_counts = np.empty(NC * NB, dtype=np.int64)
    for b in range(NC * NB):
        rows = e_srcrow_s[blk_starts[b]:blk_starts[b + 1]]
        lo_counts[b] = int((rows < HI_BASE).sum())
        hi_counts[b] = rows.shape[0] - lo_counts[b]
    sub_lo = int(np.ceil(lo_counts.max() / P))
    sub_hi = int(np.ceil(hi_counts.max() / P))
    st = sub_lo + sub_hi

    # per-core packed arrays
    idx_all = np.zeros((NC, P, NB * st * 8), dtype=np.int16)
    seg_all = np.full((NC, P, NB * st), 200.0, dtype=np.float32)
    recip_all = np.zeros((NC, P, NB), dtype=np.float32)

    recip = (1.0 / np.maximum(deg, 1)).astype(np.float32)

    def pack16(flat: np.ndarray) -> np.ndarray:
        # dma_gather layout: unwrapped[k] = tile16[k % 16, k // 16]
        n = flat.shape[0]
        t = flat.reshape(n // 16, 16).T
        return np.tile(t, (8, 1))  # [128, n/16]

    for c in range(NC):
        for lb in range(NB):
            b = c * NB + lb
            rows = e_srcrow_s[blk_starts[b]:blk_starts[b + 1]]
            segs = e_seg_s[blk_starts[b]:blk_starts[b + 1]]
            is_lo = rows < HI_BASE
            rows_lo, segs_lo = rows[is_lo], segs[is_lo]
            rows_hi, segs_hi = rows[~is_lo] - HI_BASE, segs[~is_lo]

            n_lo, n_hi = rows_lo.shape[0], rows_hi.shape[0]
            lo_pad = np.zeros(sub_lo * P, dtype=np.int16)
            lo_pad[:n_lo] = rows_lo.astype(np.int16)
            hi_pad = np.zeros(sub_hi * P, dtype=np.int16)
            hi_pad[:n_hi] = rows_hi.astype(np.int16)

            base = lb * st * 8
            idx_all[c, :, base:base + sub_lo * 8] = pack16(lo_pad)
            idx_all[c, :, base + sub_lo * 8:base + st * 8] = pack16(hi_pad)

            # segment ids: edge k of the lo list sits at partition k%128,
            # subtile k//128 (matmul contraction is the partition axis)
            sbase = lb * st
            sl = np.full(sub_lo * P, 200.0, dtype=np.float32)
            sl[:n_lo] = segs_lo.astype(np.float32)
            seg_all[c, :, sbase:sbase + sub_lo] = sl.reshape(sub_lo, P).T
            sh = np.full(sub_hi * P, 200.0, dtype=np.float32)
            sh[:n_hi] = segs_hi.astype(np.float32)
            seg_all[c, :, sbase + sub_lo:sbase + st] = sh.reshape(sub_hi, P).T

            # recip for this block's nodes
            nodes_here = np.where(block_of_node == b)[0]
            recip_all[c, pos_in_block[nodes_here], lb] = recip[nodes_here]

    return dict(
        table_row=table_row, sub_lo=sub_lo, sub_hi=sub_hi, st=st,
        idx_all=idx_all, seg_all=seg_all.astype(bf16), recip_all=recip_all,
    )


import os

N_LAYERS = int(os.environ.get("GCN_LAYERS", "3"))
NBUILD = int(os.environ.get("GCN_NBUILD", str(NB)))
USE_CC = os.environ.get("GCN_CC", "1") == "1"
USE_BIAS_MM = os.environ.get("GCN_BIAS", "1") == "1"
NO_GATHER = os.environ.get("GCN_NO_GATHER", "0") == "1"
ONE_MM = os.environ.get("GCN_ONE_MM", "0") == "1"
NO_ELU = os.environ.get("GCN_NO_ELU", "0") == "1"
NO_DENSE = os.environ.get("GCN_NO_DENSE", "0") == "1"
SINGLE_PKT = os.environ.get("GCN_SP0", "0") != "1"


def _build(sub_lo: int, sub_hi: int):
    """Build the SPMD Bass program. Returns compiled nc."""
    st = sub_lo + sub_hi
    f32 = mybir.dt.float32
    b16 = mybir.dt.bfloat16

    nc = bacc.Bacc("TRN2", target_bir_lowering=False, debug=False,
                   enable_asserts=True, num_devices=NC)

    xtab_d = nc.dram_tensor("xtab", [TAB, H], b16, kind="ExternalInput")
    xown_d = nc.dram_tensor("xown", [STRIDE, H], b16, kind="ExternalInput")
    wts_d = nc.dram_tensor("wts", [12 * P, H], b16, kind="ExternalInput")
    bias_d = nc.dram_tensor("bias", [3, H], b16, kind="ExternalInput")
    idx_d = nc.dram_tensor("idxall", [P, NB * st * 8], mybir.dt.int16,
                           kind="ExternalInput")
    seg_d = nc.dram_tensor("segall", [P, NB * st], b16, kind="ExternalInput")
    recip_d = nc.dram_tensor("recipall", [P, NB], f32, kind="ExternalInput")
    iota_d = nc.dram_tensor("iotarep", [P, st * P], b16, kind="ExternalInput")

    out_d = nc.dram_tensor("out", [STRIDE, H], f32, kind="ExternalOutput")

    with tile.TileContext(nc) as tc:
        with (
            tc.tile_pool(name="const", bufs=1) as cp,
            tc.tile_pool(name="msgp", bufs=3) as msgp,
            tc.tile_pool(name="sp", bufs=2) as sp,
            tc.tile_pool(name="actp", bufs=2) as actp,
            tc.tile_pool(name="elup", bufs=2) as elup,
            tc.tile_pool(name="pa", bufs=2, space="PSUM") as pa,
            tc.tile_pool(name="po", bufs=2, space="PSUM") as po,
            tc.tile_pool(name="ptr", bufs=2, space="PSUM") as ptr,
            tc.tile_pool(name="dram", bufs=1, space="DRAM") as dr,
        ):
            # ---- resident constants ----
            idx_t = cp.tile([P, NB * st * 8], mybir.dt.int16)
            nc.sync.dma_start(out=idx_t[:], in_=idx_d[:])
            seg_t = cp.tile([P, NB * st], b16)
            nc.sync.dma_start(out=seg_t[:], in_=seg_d[:])
            recip_t = cp.tile([P, NB], f32)
            nc.sync.dma_start(out=recip_t[:], in_=recip_d[:])
            iota_t = cp.tile([P, st * P], b16)
            nc.sync.dma_start(out=iota_t[:], in_=iota_d[:])
            wts_t = cp.tile([P, 12, H], b16)
            nc.sync.dma_start(
                out=wts_t[:], in_=wts_d[:].rearrange("(c k) h -> k c h", k=P)
            )
            if USE_BIAS_MM:
                bias_t = cp.tile([1, 3, H], b16)
                nc.sync.dma_start(
                    out=bias_t[:], in_=bias_d[:].rearrange("(a c) h -> a c h", a=1)
                )
                ones_t = cp.tile([1, P], b16)
                nc.vector.memset(ones_t[:], 1.0)
            ident_t = cp.tile([P, P], b16)
            make_identity(nc, ident_t[:])

            # internal DRAM for inter-layer activations
            h_own = [
                dr.tile([STRIDE, H], b16, tag=f"h_own{i}", name=f"h_own{i}")
                for i in range(2)
            ]
            h_full = [
                dr.tile([TAB, H], b16, tag=f"h_full{i}", name=f"h_full{i}")
                for i in range(2)
            ]

            for layer in range(N_LAYERS):
                if layer == 0:
                    tab_lo = xtab_d[:HI_BASE, :]
                    tab_hi = xtab_d[HI_BASE:, :]
                    own = xown_d
                else:
                    src_tab = h_full[layer - 1] if USE_CC else xtab_d
                    tab_lo = src_tab[:HI_BASE, :]
                    tab_hi = src_tab[HI_BASE:, :]
                    own = h_own[layer - 1]

                for b in range(NBUILD):
                    # ---- gather messages ----
                    msg = msgp.tile([P, st, H], b16, tag="msg")
                    ib = b * st * 8
                    # dma_gather dies above 1024 indices -> chunk at 8 subtiles
                    GMAX = 8
                    if NO_GATHER:
                        nc.sync.dma_start(
                            out=msg[:],
                            in_=tab_lo[0:P * st, :]
                            .rearrange("(k p) h -> p k h", p=P))
                    else:
                        for tab, s0, s1 in ((tab_lo, 0, sub_lo),
                                            (tab_hi, sub_lo, st)):
                            for g0 in range(s0, s1, GMAX):
                                g1 = min(g0 + GMAX, s1)
                                nsub = g1 - g0
                                nc.gpsimd.dma_gather(
                                    msg[:, g0:g1, :], tab,
                                    idx_t[:, ib + g0 * 8:ib + g1 * 8],
                                    nsub * P, nsub * P, H,
                                    single_packet=SINGLE_PKT,
                                )

                    # ---- one-hot ----
                    s_t = sp.tile([P, st * P], b16, tag="onehot")
                    nc.vector.tensor_tensor(
                        out=s_t[:].rearrange("p (k s) -> p k s", k=st),
                        in0=seg_t[:, b * st:(b + 1) * st].to_broadcast([P, st, P]),
                        in1=iota_t[:].rearrange("p (k s) -> p k s", k=st),
                        op=mybir.AluOpType.is_equal,
                    )

                    # ---- segmented sum ----
                    psum_agg = pa.tile([P, H], f32, tag="pagg")
                    n_mm = 1 if ONE_MM else st
                    for j in range(n_mm):
                        nc.tensor.matmul(
                            out=psum_agg[:],
                            lhsT=s_t[:, j * P:(j + 1) * P],
                            rhs=msg[:, j, :],
                            start=(j == 0),
                            stop=(j == n_mm - 1),
                        )

                    # ---- mean + self feature ----
                    agg_bf = actp.tile([P, H], b16, tag="aggbf")
                    nc.vector.tensor_scalar(
                        out=agg_bf[:], in0=psum_agg[:],
                        scalar1=recip_t[:, b:b + 1], scalar2=None,
                        op0=mybir.AluOpType.mult,
                    )
                    x_blk = actp.tile([P, H], b16, tag="xblk")
                    nc.sync.dma_start(out=x_blk[:], in_=own[b * P:(b + 1) * P, :])

                    # ---- transposes (agg | x), feature-major chunks ----
                    tr_ps = ptr.tile([P, 4, P], b16, tag="trps")
                    nc.tensor.transpose(out=tr_ps[:, 0, :], in_=agg_bf[:, 0:P],
                                        identity=ident_t[:])
                    nc.tensor.transpose(out=tr_ps[:, 1, :], in_=agg_bf[:, P:H],
                                        identity=ident_t[:])
                    nc.tensor.transpose(out=tr_ps[:, 2, :], in_=x_blk[:, 0:P],
                                        identity=ident_t[:])
                    nc.tensor.transpose(out=tr_ps[:, 3, :], in_=x_blk[:, P:H],
                                        identity=ident_t[:])
                    actT = actp.tile([P, 4, P], b16, tag="actT")
                    nc.vector.tensor_copy(out=actT[:], in_=tr_ps[:])

                    # ---- dense: out = agg @ Wl.T + x @ Wr.T + b ----
                    psum_out = po.tile([P, H], f32, tag="pout")
                    wb = layer * 4
                    for i in range(1 if NO_DENSE else 4):
                        nc.tensor.matmul(
                            out=psum_out[:],
                            lhsT=actT[:, i, :],
                            rhs=wts_t[:, wb + i, :],
                            start=(i == 0),
                            stop=(not USE_BIAS_MM and (i == 3 or NO_DENSE)),
                        )
                    if USE_BIAS_MM:
                        nc.tensor.matmul(
                            out=psum_out[:],
                            lhsT=ones_t[:],
                            rhs=bias_t[:, layer, :],
                            start=False,
                            stop=True,
                        )

                    # ---- ELU: max(z,0)-1 + exp(min(z,0)) ----
                    if NO_ELU:
                        e_t = elup.tile([P, H], f32, tag="e")
                        nc.vector.tensor_copy(out=e_t[:], in_=psum_out[:])
                        r_t = elup.tile([P, H], f32, tag="r")
                        nc.vector.tensor_copy(out=r_t[:], in_=psum_out[:])
                    else:
                        m_t = elup.tile([P, H], f32, tag="m")
                        nc.vector.tensor_scalar(
                            out=m_t[:], in0=psum_out[:], scalar1=0.0, scalar2=None,
                            op0=mybir.AluOpType.min,
                        )
                        e_t = elup.tile([P, H], f32, tag="e")
                        nc.scalar.activation(e_t[:], m_t[:],
                                             mybir.ActivationFunctionType.Exp)
                        r_t = elup.tile([P, H], f32, tag="r")
                        nc.vector.tensor_scalar(
                            out=r_t[:], in0=psum_out[:], scalar1=0.0, scalar2=-1.0,
                            op0=mybir.AluOpType.max, op1=mybir.AluOpType.add,
                        )
                    if layer < N_LAYERS - 1:
                        h_blk = elup.tile([P, H], b16, tag="hblk")
                        nc.vector.tensor_tensor(
                            out=h_blk[:], in0=r_t[:], in1=e_t[:],
                            op=mybir.AluOpType.add,
                        )
                        nc.sync.dma_start(
                            out=h_own[layer][b * P:(b + 1) * P, :], in_=h_blk[:]
                        )
                    else:
                        o_blk = elup.tile([P, H], f32, tag="oblk")
                        nc.vector.tensor_tensor(
                            out=o_blk[:], in0=r_t[:], in1=e_t[:],
                            op=mybir.AluOpType.add,
                        )
                        nc.sync.dma_start(
                            out=out_d[b * P:(b + 1) * P, :], in_=o_blk[:]
                        )

                if layer < min(2, N_LAYERS - 1) and USE_CC:
                    nc.gpsimd.collective_compute(
                        "AllGather",
                        mybir.AluOpType.bypass,
                        ins=[h_own[layer][:]],
                        outs=[h_full[layer][:]],
                        replica_groups=[list(range(NC))],
                    )

    nc.compile()
    return nc


_CACHE = {}


def _get_program(sub_lo: int, sub_hi: int):
    key = (sub_lo, sub_hi)
    if key not in _CACHE:
        _CACHE[key] = _build(sub_lo, sub_hi)
    return _CACHE[key]


def _make_in_maps(inputs: dict, pp: dict) -> list:
    x = np.asarray(inputs["x"], dtype=np.float32)
    st = pp["st"]
    table_row = pp["table_row"]

    # permuted, padded, bf16 table
    xtab = np.zeros((TAB, H), dtype=bf16)
    xtab[table_row] = x.astype(bf16)

    # weights: per layer [WlT chunk0, WlT chunk1, WrT chunk0, WrT chunk1]
    # WT[inf, outf] = W.T ; chunk k = rows [k*128, (k+1)*128)
    wchunks = []
    for l in range(3):
        for name in (f"Wl{l + 1}", f"Wr{l + 1}"):
            WT = np.asarray(inputs[name], dtype=np.float32).T.astype(bf16)
            wchunks.append(WT[0:P, :])
            wchunks.append(WT[P:H, :])
    wts = np.concatenate(wchunks, axis=0)  # [12*128, 256]

    bias = np.stack(
        [np.asarray(inputs[f"bl{l + 1}"], dtype=np.float32) for l in range(3)]
    ).astype(bf16)

    iota_rep = np.tile(np.arange(P, dtype=np.float32), (P, st)).astype(bf16)

    in_maps = []
    for c in range(NC):
        in_maps.append({
            "xtab": xtab,
            "xown": xtab[c * STRIDE:(c + 1) * STRIDE],
            "wts": wts,
            "bias": bias,
            "idxall": pp["idx_all"][c],
            "segall": pp["seg_all"][c],
            "recipall": pp["recip_all"][c],
            "iotarep": iota_rep,
        })
    return in_maps


def run(inputs: dict, trace: bool = False):
    """Returns (output [N_NODES, H] float32, exec_time_ns or None)."""
    edge_index = np.asarray(inputs["edge_index"])
    pp = _preprocess(edge_index)
    table_row = pp["table_row"]
    in_maps = _make_in_maps(inputs, pp)
    nc = _get_program(pp["sub_lo"], pp["sub_hi"])

    res = run_bass_kernel_spmd(nc, in_maps, core_ids=list(range(NC)),
                               trace=trace)

    out_full = np.empty((N_NODES, H), dtype=np.float32)
    for c in range(NC):
        shard = res.results[c]["out"]  # [STRIDE, H]
        rows = table_row - c * STRIDE
        mask = (rows >= 0) & (rows < STRIDE)
        out_full[mask] = shard[rows[mask]]
    return out_full, res.exec_time_ns


def kernel(**inputs) -> np.ndarray:
    out, _ = run(inputs)
    return out
